# revision 21
# baseline (speedup 1.0000x reference)
"""Trainium2 Bass kernel for nn_NodeInference (2-layer GAT + cosine head).

v2 design (SPMD over 8 cores, dst-node sharding, global node order):
  Tables are in GLOBAL node order; shard c owns nodes [c*CAP, (c+1)*CAP).
  Packed row tables in HBM, 384 f16 cols (768B, dma_gather elem):
     L1 row = [h0 x128 | 1 | h1 @129 x128 | 1 | pad | a_src f32 x2 @f32col130]
     L2 row = [h2 x256 | 1 | pad | a_src f32 @f32col129]
  (the embedded 1-columns let one matmul per head accumulate both the
   weighted features AND the weight sum in a single PSUM region — PE/PSUM
   accumulation groups must be consecutive single-region chains)
  P1  dense-1 SHARDED: own x shard @ W1aug -> h1loc rows; a_dst1 columns
      stashed to SBUF (win1) straight from PSUM.
  AG1 AllGather h1loc -> h1t (full table, global order).
  E1/E2 edge phase (edges grouped by dst-block; chunks of 128 edges):
      - bulk src-row dma_gather (int16 idx; lo/hi table halves @32768)
      - per-edge a_dst ON-CHIP: ones-matmul broadcasts dst_local row ->
        S^T one-hot (is_equal) -> tiny matmul S^T @ win selects a_dst
      - w'_e = exp(clamp(leakyrelu(a_s+a_d,0.2)) - C) in f16 (C per layer
        keeps w' in f16 range; cancels in the softmax division)
      - scatter: Sw[e,d] = (iota==dstloc)*w'_e built in ONE DVE op;
        bp[:,0:256] += Sw^T @ h_gathered ; bp[:,256+h] += Sw^T @ ones
      - epilogue: out = bp_h/bp_w + b (shift cancels); L1: h2aug rows ->
        cc_in + a_dst2 stash (win2)
  AG2 AllGather cc_in -> cc_out.
  Head: g^T @ h2fT; cosine vs mu -> outT [8, CAP]. Host assembles.
"""

import sys
from dataclasses import dataclass
from contextlib import ExitStack

if "/opt/trn_rl_repo" not in sys.path:
    sys.path.insert(0, "/opt/trn_rl_repo")

import numpy as np

import concourse.bacc as bacc
import concourse.bass as bass
import concourse.mybir as mybir
import concourse.tile as tile

P = 128
IN = 256          # input feature dim
HID = 256         # layer-1 output dim (2*128, concat)
OUT = 256         # layer-2 output dim
KH, MD = 8, 128   # cosine head shape
ROWW = 384        # f16 cols per packed table row (768B)
HALF = 32768      # int16 table-half split
C1, C2 = 6.0, 2.0     # per-layer exponent shifts (logits validated:
CLAMP = 10.8          # L1 e<=15.4, L2 e<=11.3; exp(CLAMP)=49021 < f16max)
AF = mybir.ActivationFunctionType
ALU = mybir.AluOpType
DT = mybir.dt
MXC = 8           # max chunks (128 idx each) per dma_gather call
AD_ZERO = False   # debug: replace on-chip a_dst select with zeros


@dataclass
class CFG:
    N: int
    W: int              # world size
    NBLK: int           # dst blocks (128 dsts) per core
    CPL: int            # lo-half chunks per block (shared by both layers)
    CPH: int

    @property
    def SHARD_CAP(self):
        return self.NBLK * P

    @property
    def CPB(self):
        return self.CPL + self.CPH

    @property
    def BPP(self):
        return (self.NBLK + 7) // 8   # dstrow blocks per partition


def build_program(cfg: CFG):
    nc = bacc.Bacc("TRN2", target_bir_lowering=False, debug=False)
    W, NBLK, CPB, CPL = cfg.W, cfg.NBLK, cfg.CPB, cfg.CPL
    CAP, BPP = cfg.SHARD_CAP, cfg.BPP
    AUG1, AUG2 = IN + 4, IN + 2
    f16, bf16, f32 = DT.float16, DT.bfloat16, DT.float32
    i16 = DT.int16

    with tile.TileContext(nc) as tc, ExitStack() as stack:
        dram = stack.enter_context(
            tc.tile_pool(name="dram", bufs=1, space="DRAM"))

        def din(name, shape, dtype):
            return dram.tile(shape, dtype, kind="ExternalInput", name=name,
                             uniquify=False)

        xTi = din("xTi", [P, NBLK, 2, P], f16)
        w1s = din("w1s", [P, 2, AUG1], f16)
        w2s = din("w2s", [P, 2, AUG2], f16)
        gsd = din("gs", [P, 2, KH * P], f16)
        mus = din("mus", [P, KH * KH], f16)       # block-diag mu^T
        ond = din("onesd", [P, KH * KH], f16)     # block-diag ones
        cmu = din("cmu", [KH, 1], f32)
        b1d = din("b1b", [P, HID], f32)
        b2d = din("b2b", [P, OUT], f32)
        iot = din("iota", [P, P], f16)            # [p,f] = f
        iop = din("iotaP", [P, 1], f32)           # [p,0] = p
        idn = din("ident", [P, P], f32)
        sl8 = din("sel8", [8, 8 * P], f16)        # sel8[k, q*P+m] = (k==q)
        onc = din("onec", [P, 1], f16)            # ones col (sumW rhs)
        isd = din("isrc", [P, NBLK * CPB * 8], i16)
        dfd = din("dstf", [P, NBLK * CPB], f32)
        drd = din("dstrow", [8, BPP * CPB * P], f16)
        outT = dram.tile([KH, CAP], f32, kind="ExternalOutput",
                         name="outT", uniquify=False)

        h1loc = dram.tile([CAP, ROWW], f16, name="h1loc")
        h1t = dram.tile([W * CAP, ROWW], f16, name="h1t",
                        addr_space="Shared" if W > 1 else "Local")
        cc_in = dram.tile([CAP, ROWW], f16, name="cc_in")
        cc_out = dram.tile([W * CAP, ROWW], f16, name="cc_out",
                           addr_space="Shared" if W > 1 else "Local")

        consts = stack.enter_context(tc.tile_pool(name="consts", bufs=1))
        x_sb = consts.tile([P, NBLK, 2, P], f16)
        w1_sb = consts.tile([P, 2, AUG1], f16)
        w2_sb = consts.tile([P, 2, AUG2], f16)
        g_sb = consts.tile([P, 2, KH * P], f16)
        mu_sb = consts.tile([P, KH * KH], f16)
        on_sb = consts.tile([P, KH * KH], f16)
        cmu_sb = consts.tile([KH, 1], f32)
        b1_sb = consts.tile([P, HID], f32)
        b2_sb = consts.tile([P, OUT], f32)
        iota_sb = consts.tile([P, P], f16)
        iotaP_sb = consts.tile([P, 1], f32)
        ident_sb = consts.tile([P, P], f32)
        sel8_sb = consts.tile([8, 8 * P], f16)
        onec_sb = consts.tile([P, 1], f16)
        isrc_sb = consts.tile([P, NBLK * CPB * 8], i16)
        dstf_sb = consts.tile([P, NBLK * CPB], f32)
        drow_sb = consts.tile([8, BPP * CPB * P], f16)
        win1_sb = consts.tile([P, NBLK * 2], f16)
        win2_sb = consts.tile([P, NBLK], f16)
        out1T_sb = consts.tile([P, 2, CAP], f16)
        h2fT_sb = consts.tile([P, 2, CAP], f16)

        for dst, src in [(x_sb, xTi), (w1_sb, w1s), (w2_sb, w2s),
                         (g_sb, gsd), (mu_sb, mus), (on_sb, ond),
                         (cmu_sb, cmu), (b1_sb, b1d), (b2_sb, b2d),
                         (iota_sb, iot), (iotaP_sb, iop), (ident_sb, idn),
                         (sel8_sb, sl8), (onec_sb, onc), (isrc_sb, isd),
                         (dstf_sb, dfd), (drow_sb, drd)]:
            nc.sync.dma_start(dst[:], src[:])

        # ================= P1: dense layer 1 (own shard only) ================
        with tc.tile_pool(name="p1ps", bufs=2, space="PSUM") as p1ps, \
             tc.tile_pool(name="p1row", bufs=3) as p1row:
            for t in range(NBLK):
                ps = p1ps.tile([P, AUG1], f32, tag="ps")
                for k in range(2):
                    nc.tensor.matmul(ps[:], lhsT=x_sb[:, t, k, :],
                                     rhs=w1_sb[:, k, :],
                                     start=(k == 0), stop=(k == 1))
                nc.vector.tensor_copy(win1_sb[:, 2 * t:2 * t + 2],
                                      ps[:, IN + 2:IN + 4])
                row = p1row.tile([P, ROWW], f16, tag="row")
                nc.scalar.activation(row[:, 0:P], ps[:, 0:P], AF.Copy)
                nc.scalar.activation(row[:, P + 1:IN + 1], ps[:, P:IN],
                                     AF.Copy)
                nc.vector.memset(row[:, P:P + 1], 1.0)
                nc.vector.memset(row[:, IN + 1:IN + 2], 1.0)
                rf32 = row[:].bitcast(f32)
                nc.vector.tensor_copy(rf32[:, 130:132], ps[:, IN:IN + 2])
                nc.sync.dma_start(h1loc[t * P:(t + 1) * P, 0:IN + 8],
                                  row[:, 0:IN + 8])

        if W > 1:
            nc.gpsimd.collective_compute(
                "AllGather", ALU.bypass,
                replica_groups=[list(range(W))],
                ins=[h1loc[:]], outs=[h1t[:]])
        else:
            nc.sync.dma_start(h1t[0:CAP, :], h1loc[:])

        # ================= E1/E2: edge phases ================================
        def edge_phase(layer):
            if layer == 1:
                table, win_sb = h1t, win1_sb
                nh, Cshift = 2, C1
                b_sb, out_t, lrelu_out = b1_sb, out1T_sb, True
                HW, as_off = P + 1, 130   # head width incl 1-col; a_src f32
            else:
                table, win_sb = cc_out, win2_sb
                nh, Cshift = 1, C2
                b_sb, out_t, lrelu_out = b2_sb, h2fT_sb, False
                HW, as_off = IN + 1, 129

            estack = ExitStack()
            pg = estack.enter_context(
                tc.tile_pool(name=f"gath{layer}", bufs=2))
            pst = estack.enter_context(
                tc.tile_pool(name=f"bc{layer}", bufs=1, space="PSUM"))
            pstt = estack.enter_context(tc.tile_pool(name=f"st{layer}",
                                                     bufs=2))
            pad_ = estack.enter_context(
                tc.tile_pool(name=f"adps{layer}", bufs=1, space="PSUM"))
            pew = estack.enter_context(tc.tile_pool(name=f"ew{layer}",
                                                    bufs=2))
            psw = estack.enter_context(tc.tile_pool(name=f"sw{layer}",
                                                    bufs=2))
            pbp = estack.enter_context(
                tc.tile_pool(name=f"bps{layer}", bufs=2, space="PSUM"))
            pt = estack.enter_context(
                tc.tile_pool(name=f"tps{layer}", bufs=1, space="PSUM"))
            po = estack.enter_context(tc.tile_pool(name=f"epi{layer}",
                                                   bufs=2))
            ph = estack.enter_context(
                tc.tile_pool(name=f"h2ps{layer}", bufs=1, space="PSUM"))

            nrow = table.shape[0]
            tab_lo = table[0:min(HALF, nrow), :]
            tab_hi = table[HALF:nrow, :] if nrow > HALF else tab_lo

            for blk in range(NBLK):
                base8 = blk * CPB * 8
                basec = blk * CPB
                dro = (blk // 8) * CPB * P
                prow = blk % 8

                gt = pg.tile([P, CPB, ROWW], f16, tag="gt")
                if blk < 2:
                    # pool buffers start uninitialized; pad slots must hold
                    # finite f16 so exp() of stale a_src can't poison via NaN
                    nc.vector.memset(gt[:], 0.0)
                for c0 in range(0, CPL, MXC):
                    c1 = min(c0 + MXC, CPL)
                    nc.gpsimd.dma_gather(
                        gt[:, c0:c1, :], tab_lo,
                        isrc_sb[:, base8 + c0 * 8:base8 + c1 * 8],
                        (c1 - c0) * P, (c1 - c0) * P, ROWW)
                for c0 in range(CPL, CPB, MXC):
                    c1 = min(c0 + MXC, CPB)
                    nc.gpsimd.dma_gather(
                        gt[:, c0:c1, :], tab_hi,
                        isrc_sb[:, base8 + c0 * 8:base8 + c1 * 8],
                        (c1 - c0) * P, (c1 - c0) * P, ROWW)

                # ---- per-edge a_dst via on-chip select -----------------
                # bc[p, e] = dst_local(edge e) for all p (sel8 one-hot picks
                # the dstrow partition holding this block); S^T = (bc == p);
                # ad[e, :] = S^T(d,e)^T @ win(d, :)
                ad_ps = pad_.tile([P, CPB, nh], f32, tag="adps")
                if AD_ZERO:
                    nc.vector.memset(ad_ps[:], 0.0)
                for g0 in ([] if AD_ZERO else range(0, CPB, 4)):
                    g1 = min(g0 + 4, CPB)
                    gw = (g1 - g0) * P
                    bc = pst.tile([P, 4 * P], f32, tag="bc")
                    nc.tensor.matmul(
                        bc[:, 0:gw],
                        lhsT=sel8_sb[:, prow * P:(prow + 1) * P],
                        rhs=drow_sb[:, dro + g0 * P:dro + g1 * P],
                        start=True, stop=True)
                    st = pstt.tile([P, 4 * P], f16, tag="st")
                    nc.vector.tensor_scalar(
                        out=st[:, 0:gw], in0=bc[:, 0:gw],
                        scalar1=iotaP_sb[:, 0:1], scalar2=None,
                        op0=ALU.is_equal)
                    for j in range(g0, g1):
                        nc.tensor.matmul(
                            ad_ps[:, j, :],
                            lhsT=st[:, (j - g0) * P:(j - g0 + 1) * P],
                            rhs=win_sb[:, blk * nh:(blk + 1) * nh],
                            start=True, stop=True)

                # ---- edge weights w' = exp(clamp(lrelu(as+ad)) - C) ----
                gtf = gt[:].bitcast(f32)
                as_v = gtf[:, :, as_off:as_off + nh]
                ew = pew.tile([P, CPB, nh], f32, tag="ew")
                tl = pew.tile([P, CPB, nh], f32, tag="tl")
                wv = pew.tile([P, CPB, nh], f32, tag="wv")
                nc.vector.tensor_tensor(ew[:], as_v, ad_ps[:], op=ALU.add)
                nc.vector.tensor_scalar(out=tl[:], in0=ew[:], scalar1=0.2,
                                        scalar2=None, op0=ALU.mult)
                nc.vector.tensor_tensor(tl[:], tl[:], ew[:], op=ALU.max)
                nc.vector.tensor_scalar(out=tl[:], in0=tl[:],
                                        scalar1=CLAMP + Cshift,
                                        scalar2=Cshift,
                                        op0=ALU.min, op1=ALU.subtract)
                nc.scalar.activation(wv[:], tl[:], AF.Exp)

                # ---- scatter: per head, ONE consecutive accumulation
                # group bp[:, h*HW:(h+1)*HW] += Sw_j^T @ [h_j | 1] over j
                swa = psw.tile([P, CPB, nh, P], f16, tag="swa")
                for j in range(CPB):
                    for h in range(nh):
                        nc.vector.tensor_scalar(
                            out=swa[:, j, h, :], in0=iota_sb[:],
                            scalar1=dstf_sb[:, basec + j:basec + j + 1],
                            scalar2=wv[:, j, h:h + 1],
                            op0=ALU.is_equal, op1=ALU.mult)
                bp = pbp.tile([P, nh * HW], f32, tag="bp")
                for h in range(nh):
                    for j in range(CPB):
                        nc.tensor.matmul(bp[:, h * HW:(h + 1) * HW],
                                         lhsT=swa[:, j, h, :],
                                         rhs=gt[:, j, h * HW:(h + 1) * HW],
                                         start=(j == 0), stop=(j == CPB - 1))

                # ---- block epilogue ------------------------------------
                rec = po.tile([P, nh], f32, tag="rec")
                ti = po.tile([P, IN], f32, tag="ti")
                for h in range(nh):
                    nc.vector.reciprocal(rec[:, h:h + 1],
                                         bp[:, h * HW + HW - 1:(h + 1) * HW])
                hw0 = HW - 1
                if nh == 2:
                    nc.vector.tensor_scalar(out=ti[:, 0:P], in0=bp[:, 0:hw0],
                                            scalar1=rec[:, 0:1],
                                            scalar2=None, op0=ALU.mult)
                    nc.vector.tensor_scalar(out=ti[:, P:IN],
                                            in0=bp[:, HW:HW + hw0],
                                            scalar1=rec[:, 1:2],
                                            scalar2=None, op0=ALU.mult)
                else:
                    nc.vector.tensor_scalar(out=ti[:], in0=bp[:, 0:hw0],
                                            scalar1=rec[:, 0:1],
                                            scalar2=None, op0=ALU.mult)
                nc.vector.tensor_tensor(ti[:], ti[:], b_sb[:], op=ALU.add)
                if lrelu_out:
                    tm = po.tile([P, IN], f32, tag="tm")
                    nc.vector.tensor_scalar(out=tm[:], in0=ti[:],
                                            scalar1=0.01, scalar2=None,
                                            op0=ALU.mult)
                    nc.vector.tensor_tensor(ti[:], tm[:], ti[:], op=ALU.max)
                for k in range(2):
                    tp = pt.tile([P, P], f32, tag="tp")
                    nc.tensor.transpose(tp[:], ti[:, k * P:(k + 1) * P],
                                        ident_sb[:])
                    dsl = out_t[:, k, blk * P:(blk + 1) * P]
                    if k == 0:
                        nc.scalar.activation(dsl, tp[:], AF.Copy)
                    else:
                        nc.vector.tensor_copy(dsl, tp[:])

                if layer == 1:
                    hp = ph.tile([P, AUG2], f32, tag="hp")
                    for k in range(2):
                        nc.tensor.matmul(
                            hp[:],
                            lhsT=out1T_sb[:, k, blk * P:(blk + 1) * P],
                            rhs=w2_sb[:, k, :], start=(k == 0), stop=(k == 1))
                    nc.vector.tensor_copy(win2_sb[:, blk:blk + 1],
                                          hp[:, OUT + 1:OUT + 2])
                    row2 = po.tile([P, ROWW], f16, tag="row2")
                    nc.scalar.activation(row2[:, 0:OUT], hp[:, 0:OUT],
                                         AF.Copy)
                    nc.vector.memset(row2[:, OUT:OUT + 1], 1.0)
                    r2f = row2[:].bitcast(f32)
                    nc.vector.tensor_copy(r2f[:, 129:130],
                                          hp[:, OUT:OUT + 1])
                    nc.sync.dma_start(
                        cc_in[blk * P:(blk + 1) * P, 0:OUT + 4],
                        row2[:, 0:OUT + 4])

            estack.close()

        edge_phase(1)

        if W > 1:
            nc.gpsimd.collective_compute(
                "AllGather", ALU.bypass,
                replica_groups=[list(range(W))],
                ins=[cc_in[:]], outs=[cc_out[:]])
        else:
            nc.sync.dma_start(cc_out[0:CAP, :], cc_in[:])

        edge_phase(2)

        # ================= head: cosine vs mu ================================
        with tc.tile_pool(name="hps", bufs=2, space="PSUM") as hps, \
             tc.tile_pool(name="hsb", bufs=3) as hsb, \
             tc.tile_pool(name="sps", bufs=2, space="PSUM") as sps, \
             tc.tile_pool(name="hepi", bufs=2) as hepi:
            st = 0
            while st < CAP:
                wdt = min(512, CAP - st)
                nump = sps.tile([KH, 512], f32, tag="nump")
                nrmp = sps.tile([KH, 512], f32, tag="nrmp")
                for k in range(KH):
                    hp = hps.tile([P, 512], f32, tag="hp")
                    for f in range(2):
                        nc.tensor.matmul(hp[:, 0:wdt],
                                         lhsT=g_sb[:, f, k * P:(k + 1) * P],
                                         rhs=h2fT_sb[:, f, st:st + wdt],
                                         start=(f == 0), stop=(f == 1))
                    h16 = hsb.tile([P, 512], f16, tag="h16")
                    sq16 = hsb.tile([P, 512], f16, tag="sq16")
                    nc.vector.tensor_copy(h16[:, 0:wdt], hp[:, 0:wdt])
                    nc.scalar.activation(sq16[:, 0:wdt], hp[:, 0:wdt],
                                         AF.Square)
                    nc.tensor.matmul(nump[:, 0:wdt],
                                     lhsT=mu_sb[:, k * KH:(k + 1) * KH],
                                     rhs=h16[:, 0:wdt], start=(k == 0),
                                     stop=(k == KH - 1))
                    nc.tensor.matmul(nrmp[:, 0:wdt],
                                     lhsT=on_sb[:, k * KH:(k + 1) * KH],
                                     rhs=sq16[:, 0:wdt], start=(k == 0),
                                     stop=(k == KH - 1))
                sq = hepi.tile([KH, 512], f32, tag="sqr")
                nc.scalar.activation(sq[:, 0:wdt], nrmp[:, 0:wdt], AF.Sqrt)
                nc.vector.tensor_scalar(out=sq[:, 0:wdt], in0=sq[:, 0:wdt],
                                        scalar1=cmu_sb[:, 0:1], scalar2=1e-8,
                                        op0=ALU.mult, op1=ALU.max)
                nc.vector.reciprocal(sq[:, 0:wdt], sq[:, 0:wdt])
                res = hepi.tile([KH, 512], f32, tag="res")
                nc.vector.tensor_tensor(res[:, 0:wdt], nump[:, 0:wdt],
                                        sq[:, 0:wdt], op=ALU.mult)
                nc.sync.dma_start(outT[:, st:st + wdt], res[:, 0:wdt])
                st += wdt

    nc.compile()
    return nc


# ======================= host-side preparation ==============================

def _wrap16(flat):
    """idx flat [n] -> wrapped int16 [128, n//16]; pos i -> (i%16, i//16),
    replicated across the 8 Q7-core stripes."""
    n = len(flat)
    out = np.zeros((P, n // 16), np.int16)
    cols = np.arange(n) // 16
    rows = np.arange(n) % 16
    for r in range(8):
        out[r * 16 + rows, cols] = flat
    return out


def prep_host(x, edge_index, W1, a_src1, a_dst1, b1, W2, a_src2, a_dst2, b2,
              g, mu, world=8):
    x = np.asarray(x, np.float32)
    N = x.shape[0]
    NBLK = int(np.ceil(N / world / P))
    CAP = NBLK * P

    src = np.concatenate([np.asarray(edge_index[0]),
                          np.arange(N)]).astype(np.int64)
    dst = np.concatenate([np.asarray(edge_index[1]),
                          np.arange(N)]).astype(np.int64)
    core = np.minimum(dst // CAP, world - 1)
    blk = (dst - core * CAP) // P

    gkey = core * NBLK + blk
    gorder = np.argsort(gkey, kind="stable")
    src, dst, gkey = src[gorder], dst[gorder], gkey[gorder]
    starts = np.concatenate(
        [[0], np.cumsum(np.bincount(gkey, minlength=world * NBLK))])

    ed = {}
    CPL = CPH = 1
    for c in range(world):
        for b in range(NBLK):
            gid = c * NBLK + b
            es = src[starts[gid]:starts[gid + 1]]
            eds = dst[starts[gid]:starts[gid + 1]]
            dloc = (eds - c * CAP - b * P).astype(np.int64)
            lo = es < HALF
            ed[(c, b)] = (es, lo, dloc)
            CPL = max(CPL, int(np.ceil(lo.sum() / P)))
            CPH = max(CPH, int(np.ceil((~lo).sum() / P)))

    cfg = CFG(N=N, W=world, NBLK=NBLK, CPL=CPL, CPH=CPH)
    CPB, BPP = cfg.CPB, cfg.BPP

    def build_edges(c):
        isrc = np.zeros((P, NBLK * CPB * 8), np.int16)
        dstf = np.full((P, NBLK * CPB), -1.0, np.float32)
        drow = np.full((8, BPP * CPB * P), -1.0, np.float16)
        for b in range(NBLK):
            ids, lo, dloc = ed[(c, b)]
            fl = np.zeros(CPB * P, np.int64)     # slot -> table idx (pad 0)
            fd = np.full(CPB * P, -1, np.int64)  # slot -> dst_local (pad -1)
            ilo = np.where(lo)[0]
            ihi = np.where(~lo)[0]
            fl[:len(ilo)] = ids[ilo]
            fd[:len(ilo)] = dloc[ilo]
            fl[CPL * P:CPL * P + len(ihi)] = ids[ihi] - HALF
            fd[CPL * P:CPL * P + len(ihi)] = dloc[ihi]
            isrc[:, b * CPB * 8:(b + 1) * CPB * 8] = _wrap16(fl)
            # dstf[p, b*CPB + j] = fd[j*128 + p]
            dstf[:, b * CPB:(b + 1) * CPB] = \
                fd.reshape(CPB, P).T.astype(np.float32)
            dro = (b // 8) * CPB * P
            drow[b % 8, dro:dro + CPB * P] = fd.astype(np.float16)
        return isrc, dstf, drow

    # weights
    W1 = np.asarray(W1, np.float32)
    W2 = np.asarray(W2, np.float32)
    W1r = W1.reshape(2, MD, IN)
    Ps1 = np.einsum("hdi,hd->ih", W1r, np.asarray(a_src1, np.float32))
    Pd1 = np.einsum("hdi,hd->ih", W1r, np.asarray(a_dst1, np.float32))
    W1aug = np.concatenate([W1.T, Ps1, Pd1], axis=1)
    Ps2 = W2.T @ np.asarray(a_src2, np.float32)[0][:, None]
    Pd2 = W2.T @ np.asarray(a_dst2, np.float32)[0][:, None]
    W2aug = np.concatenate([W2.T, Ps2, Pd2], axis=1)
    AUG1, AUG2 = IN + 4, IN + 2
    w1s = W1aug.reshape(2, P, AUG1).transpose(1, 0, 2).astype(np.float16)
    w2s = W2aug.reshape(2, P, AUG2).transpose(1, 0, 2).astype(np.float16)

    gm = np.asarray(g, np.float32)
    gsd = gm.reshape(2, P, KH * P).transpose(1, 0, 2).astype(np.float16)
    mu = np.asarray(mu, np.float32)
    mus = np.zeros((P, KH * KH), np.float16)
    onesd = np.zeros((P, KH * KH), np.float16)
    for k in range(KH):
        mus[:, k * KH + k] = mu[k, :]
        onesd[:, k * KH + k] = 1.0
    cmu = np.linalg.norm(mu, axis=1)[:, None].astype(np.float32)
    b1b = np.broadcast_to(np.asarray(b1, np.float32), (P, HID)).copy()
    b2b = np.broadcast_to(np.asarray(b2, np.float32), (P, OUT)).copy()
    iota = np.broadcast_to(np.arange(P, dtype=np.float32),
                           (P, P)).astype(np.float16)
    iotaP = np.arange(P, dtype=np.float32)[:, None].copy()
    ident = np.eye(P, dtype=np.float32)
    sel8 = np.zeros((8, 8 * P), np.float16)
    for q in range(8):
        sel8[q, q * P:(q + 1) * P] = 1.0
    onec = np.ones((P, 1), np.float16)

    shared = dict(w1s=w1s, w2s=w2s, gs=gsd, mus=mus, onesd=onesd, cmu=cmu,
                  b1b=b1b, b2b=b2b, iota=iota, iotaP=iotaP, ident=ident,
                  sel8=sel8, onec=onec)
    in_maps = []
    for c in range(world):
        xp = np.zeros((CAP, IN), np.float32)
        n = min(CAP, N - c * CAP)
        xp[:n] = x[c * CAP:c * CAP + n]
        xTi = xp.reshape(NBLK, P, 2, P).transpose(3, 0, 2, 1)\
            .astype(np.float16)
        i1, f1, r1 = build_edges(c)
        m = dict(shared)
        m.update(xTi=xTi, isrc=i1, dstf=f1, dstrow=r1)
        in_maps.append(m)
    return cfg, in_maps


_CACHE = {}


def kernel(**inputs):
    world = 8
    cfg, in_maps = prep_host(world=world, **inputs)
    key = (cfg.N, cfg.W, cfg.CPB)
    if key not in _CACHE:
        _CACHE[key] = build_program(cfg)
    nc = _CACHE[key]

    from concourse.bass_utils import run_bass_kernel_spmd
    res = run_bass_kernel_spmd(nc, in_maps, core_ids=list(range(world)))
    outs = res.results
    N, CAP = cfg.N, cfg.SHARD_CAP
    full = np.zeros((N, KH), np.float32)
    for c in range(world):
        o = outs[c]["outT"]
        n = min(CAP, N - c * CAP)
        full[c * CAP:c * CAP + n, :] = o[:, :n].T
    return full
